# revision 1
# baseline (speedup 1.0000x reference)
"""Trainium2 Bass kernel for nn_DocEncoder (Fastformer doc encoder).

Strategy: data-parallel over batch across 8 NeuronCores (512 docs/core).
Per core, activations are kept feature-major ([features-on-partitions,
tokens-on-free]) so every GEMM and every per-document (segment of S=64
tokens along the free dim) reduction maps cleanly onto PE / DVE / ACT.

The embedding lookup uses the GPSIMD dma_gather extended instruction in
transpose mode (gather + transpose fused), with the vocabulary rebased at
row 25000 so the int16 gather indices cover the 50k vocab via their signed
range.  Head-wise softmaxes are computed normalization-free (exp, then a
per-doc reciprocal-of-sum rescale of the tiny doc-space accumulators).
"""

import os
import sys

import numpy as np
import ml_dtypes

sys.path.insert(0, "/opt/trn_rl_repo")

bf16 = ml_dtypes.bfloat16

# problem constants
B, S, V, D, H, E, VS = 4096, 64, 50000, 300, 6, 400, 200
DH = 50
SCALE = DH ** -0.5
NCORES = 8
BP = B // NCORES          # 512 docs per core
TOK = BP * S              # 32768 tokens per core
MACRO = 2048              # tokens per macro tile (32 docs)
NMACRO = TOK // MACRO     # 16
DOCS_M = MACRO // S       # 32 docs per macro
PADI = 128                # pad indices per gather (avoid trailing-negative trim)
GIDX = MACRO + PADI       # 2176
SUB = 512                 # matmul free-dim tile
NSUB = MACRO // SUB       # 4
EPAD = 384                # padded emb row length (3 x 128)
REBASE = 25000            # vocab rebase so indices fit int16

_CACHE = {}


def _fold_weights(t):
    """Host-side weight folding. Returns dict of device arrays."""
    f32 = np.float32
    Wq, Wk, Wv = [np.asarray(t[n], f32) for n in ("Wq", "Wk", "Wv")]
    wq_attn, wk_attn = np.asarray(t["wq_attn"], f32), np.asarray(t["wk_attn"], f32)
    Wr, br = np.asarray(t["Wr"], f32), np.asarray(t["br"], f32)
    Wo, bo = np.asarray(t["Wo"], f32), np.asarray(t["bo"], f32)
    Wp, bp = np.asarray(t["Wp"], f32), np.asarray(t["bp"], f32)
    Wa, ba, va = np.asarray(t["Wa"], f32), np.asarray(t["ba"], f32), np.asarray(t["va"], f32)

    Wqa = np.zeros((D, H), f32)
    Wka = np.zeros((D, H), f32)
    for h in range(H):
        Wqa[h * DH:(h + 1) * DH, h] = wq_attn[h]
        Wka[h * DH:(h + 1) * DH, h] = wk_attn[h]
    Aq = (Wq @ Wqa) * SCALE            # [300, 6], applied to x
    Ka6 = Wka * SCALE                  # [300, 6], applied to p
    Wr_blk = np.zeros((D, D), f32)
    for h in range(H):
        Wr_blk[h * DH:(h + 1) * DH, h * DH:(h + 1) * DH] = Wr
    Wop = Wo @ Wp
    Wup = Wr_blk @ Wop                 # [300, 400]
    Wqp = Wop                          # [300, 400]
    bh = np.tile(br, H) @ Wop + bo @ Wp + bp   # [400]

    def k128(w):
        # [300, M] -> [128, 3, M] zero-padded K chunks of 128 (input features)
        out = np.zeros((128, 3, w.shape[1]), f32)
        for c in range(3):
            rows = w[c * 128:min((c + 1) * 128, D)]
            out[:rows.shape[0], c] = rows
        return out.astype(bf16)

    def k100(w):
        # [300, M] -> [128, 3, M] zero-padded K chunks of 100
        out = np.zeros((128, 3, w.shape[1]), f32)
        for c in range(3):
            rows = w[c * 100:(c + 1) * 100]
            out[:rows.shape[0], c] = rows
        return out.astype(bf16)

    def k100_4(w):
        # [400, M] -> [128, 4, M] zero-padded K chunks of 100
        out = np.zeros((128, 4, w.shape[1]), f32)
        for c in range(4):
            out[:100, c] = w[c * 100:(c + 1) * 100]
        return out.astype(bf16)

    # head indicator for E replication: b6a[h, d] = 1 if d // 50 == h (d in 0..299)
    b6a = np.zeros((H, D), f32)
    for d in range(D):
        b6a[d // DH, d] = 1.0
    dev = {
        "wq": k128(Wq), "wk": k128(Wk), "wv": k128(Wv),
        "aq6": k128(Aq),
        "ka6": k100(Ka6),
        "wup": k100(Wup), "wqp": k100(Wqp),
        "wa": k100_4(Wa),
        "b6a": b6a.astype(bf16),                       # [6, 300]
        "b6a_f": b6a.astype(f32),                      # [6, 300] fp32
        "on1_f": np.ones((1, 128), f32),
        "va2": k100(va.reshape(VS, 1))[:, :2],         # [128, 2, 1]
        "on1": np.ones((1, 128), bf16),                # [1, 128]
        "bh": np.stack([bh[c * 100:(c + 1) * 100] for c in range(4)], 1)
                .astype(f32),                          # [100, 4]
        "ba2": np.stack([ba[c * 100:(c + 1) * 100] for c in range(2)], 1)
                .astype(f32),                          # [100, 2]
    }
    return dev


def _build_program(stages=99):
    import concourse.bass as bass
    import concourse.bacc as bacc
    import concourse.mybir as mybir
    from concourse import library_config
    from concourse.tile import TileContext

    fp32 = mybir.dt.float32
    bft = mybir.dt.bfloat16
    MULT = mybir.AluOpType.mult
    ADD = mybir.AluOpType.add
    AF = mybir.ActivationFunctionType

    nc = bacc.Bacc(None, target_bir_lowering=False)

    # DRAM tensors
    embp = nc.dram_tensor("embp", [V, EPAD], bft, kind="ExternalInput")
    idx = nc.dram_tensor("idx", [128, NMACRO * (GIDX // 16)], mybir.dt.int16,
                         kind="ExternalInput")
    w_dram = {}
    for name, shape, dt in [
        ("wq", [128, 3, D], bft), ("wk", [128, 3, D], bft), ("wv", [128, 3, D], bft),
        ("aq6", [128, 3, H], bft), ("ka6", [128, 3, H], bft),
        ("wup", [128, 3, E], bft), ("wqp", [128, 3, E], bft),
        ("wa", [128, 4, VS], bft),
        ("b6a", [H, D], bft), ("va2", [128, 2, 1], bft), ("on1", [1, 128], bft),
        ("b6a_f", [H, D], fp32), ("on1_f", [1, 128], fp32),
        ("bh", [100, 4], fp32), ("ba2", [100, 2], fp32),
    ]:
        w_dram[name] = nc.dram_tensor(name, shape, dt, kind="ExternalInput")
    outp = nc.dram_tensor("outp", [128, 4, BP], fp32, kind="ExternalOutput")

    with TileContext(nc) as tc:
        with (
            tc.tile_pool(name="wpool", bufs=1) as wpool,
            tc.tile_pool(name="xpool", bufs=2) as xpool,
            tc.tile_pool(name="act", bufs=1) as act_pool,
            tc.tile_pool(name="act2", bufs=1) as act2_pool,
            tc.tile_pool(name="rep", bufs=2) as rep_pool,
            tc.tile_pool(name="scr", bufs=2) as scr_pool,
            tc.tile_pool(name="small", bufs=2) as small_pool,
            tc.tile_pool(name="acc", bufs=1) as acc_pool,
            tc.tile_pool(name="ps", bufs=6, space="PSUM") as ps_pool,
            tc.tile_pool(name="ps_s", bufs=2, space="PSUM") as ps_s_pool,
        ):
            # ---- load weights + indices into SBUF (once) ----
            wsb = {}
            for name, t in w_dram.items():
                tile = wpool.tile(t.shape, t.dtype, tag=name)
                nc.sync.dma_start(out=tile[:], in_=t[:])
                wsb[name] = tile
            idx_sb = wpool.tile([128, NMACRO * (GIDX // 16)], mybir.dt.int16,
                                tag="idx")
            nc.sync.dma_start(out=idx_sb[:], in_=idx[:])

            pooled_acc = acc_pool.tile([128, 4, BP], fp32, tag="pooled")
            if stages < 6:
                nc.vector.memset(pooled_acc[:], 0.0)

            nc.gpsimd.load_library(library_config.mlp)

            emb_re = embp[REBASE:, :]  # rebased gather base

            for m in range(NMACRO):
                # ---- gather: xT [128, 3, GIDX] bf16, feature-major ----
                xT = xpool.tile([128, 3, GIDX], bft, tag="xT")
                icols = GIDX // 16
                nc.gpsimd.dma_gather(
                    out_ap=xT[:],
                    in_ap=emb_re,
                    idxs_ap=idx_sb[:, m * icols:(m + 1) * icols],
                    num_idxs=GIDX,
                    num_idxs_reg=GIDX,
                    elem_size=EPAD,
                    transpose=True,
                    single_packet=False,
                )

                if stages < 2:
                    continue
                q_sb = act2_pool.tile([128, 3, MACRO], bft, tag="q")
                k_sb = act2_pool.tile([128, 3, MACRO], bft, tag="k")
                v_sb = act_pool.tile([128, 3, MACRO], bft, tag="v")
                e_sb = act_pool.tile([6, MACRO], bft, tag="ea")

                # ---- qkv + alpha logits (PE), PSUM -> SBUF via ACT ----
                for s in range(NSUB):
                    sl = slice(s * SUB, (s + 1) * SUB)
                    for wname, dst in (("wq", q_sb), ("wk", k_sb), ("wv", v_sb)):
                        for mo in range(3):
                            ps = ps_pool.tile([128, SUB], fp32, tag="mm")
                            for ki in range(3):
                                nc.tensor.matmul(
                                    ps[:100, :],
                                    lhsT=wsb[wname][:, ki, mo * 100:(mo + 1) * 100],
                                    rhs=xT[:, ki, sl],
                                    start=(ki == 0), stop=(ki == 2),
                                )
                            nc.scalar.activation(
                                out=dst[:100, mo, sl], in_=ps[:100, :], func=AF.Copy)
                    ps6 = ps_pool.tile([128, SUB], fp32, tag="mm")
                    for ki in range(3):
                        nc.tensor.matmul(
                            ps6[:6, :], lhsT=wsb["aq6"][:, ki, :], rhs=xT[:, ki, sl],
                            start=(ki == 0), stop=(ki == 2))
                    nc.scalar.activation(out=e_sb[:6, sl], in_=ps6[:6, :], func=AF.Exp)

                if stages < 3:
                    continue
                # ---- Za, rZa [6, 32] ----
                za = small_pool.tile([6, DOCS_M], fp32, tag="za")
                nc.vector.tensor_reduce(
                    out=za[:], in_=e_sb[:6, :].rearrange("p (b s) -> p b s", s=S),
                    axis=mybir.AxisListType.X, op=ADD)
                rza = small_pool.tile([6, DOCS_M], fp32, tag="rza")
                nc.vector.reciprocal(out=rza[:], in_=za[:])

                def seg_wsum(dst, src_t, chunks, rep_src, tag):
                    """dst[:100, c, :DOCS_M] (fp32) = sum_s(rep(c) * src)."""
                    for c in range(chunks):
                        er = rep_src(c)
                        ta = scr_pool.tile([128, MACRO], bft, tag="ta")
                        nc.vector.tensor_tensor(
                            out=ta[:100, :], in0=src_t[:100, c, :], in1=er[:100, :],
                            op=MULT)
                        t3 = ta[:100, :].rearrange("p (b s) -> p b s", s=S)
                        f1 = scr_pool.tile([128, DOCS_M, 32], bft, tag="f1")
                        nc.vector.tensor_tensor(
                            out=f1[:100], in0=t3[:, :, 0:32], in1=t3[:, :, 32:64],
                            op=ADD)
                        f2 = scr_pool.tile([128, DOCS_M, 16], bft, tag="f2")
                        nc.vector.tensor_tensor(
                            out=f2[:100], in0=f1[:100, :, 0:16], in1=f1[:100, :, 16:32],
                            op=ADD)
                        f3 = scr_pool.tile([128, DOCS_M, 8], bft, tag="f3")
                        nc.vector.tensor_tensor(
                            out=f3[:100], in0=f2[:100, :, 0:8], in1=f2[:100, :, 8:16],
                            op=ADD)
                        nc.vector.tensor_reduce(
                            out=dst[:100, c, :], in_=f3[:100],
                            axis=mybir.AxisListType.X, op=ADD)

                def make_rep(e_src, tag):
                    """c -> SBUF bf16 [128, MACRO]: per-head [6, *] replicated to
                    the features of chunk c (K=6 PE matmul + ACT copy)."""
                    def rep(c):
                        er = rep_pool.tile([128, MACRO], bft, tag=tag)
                        for s in range(NSUB):
                            sl = slice(s * SUB, (s + 1) * SUB)
                            psE = ps_pool.tile([128, SUB], fp32, tag="mm")
                            nc.tensor.matmul(
                                psE[:100, :],
                                lhsT=wsb["b6a"][:, c * 100:(c + 1) * 100],
                                rhs=e_src[:6, sl], start=True, stop=True)
                            nc.scalar.activation(
                                out=er[:100, sl], in_=psE[:100, :], func=AF.Copy)
                        return er
                    return rep

                # ---- g ----
                g_un = small_pool.tile([128, 3, DOCS_M], fp32, tag="gun")
                seg_wsum(g_un, q_sb, 3, make_rep(e_sb, "erep"), "a")
                g_sb = small_pool.tile([128, 3, DOCS_M], bft, tag="g")
                for c in range(3):
                    psZ = ps_s_pool.tile([128, DOCS_M], fp32, tag="mmz")
                    nc.tensor.matmul(
                        psZ[:100, :], lhsT=wsb["b6a_f"][:, c * 100:(c + 1) * 100],
                        rhs=rza[:], start=True, stop=True)
                    nc.vector.tensor_tensor(
                        out=g_sb[:100, c, :], in0=g_un[:100, c, :], in1=psZ[:100, :],
                        op=MULT)

                if stages < 4:
                    continue
                # ---- p = g (bcast over s) * k ----
                p_sb = act_pool.tile([128, 3, MACRO], bft, tag="p")
                for c in range(3):
                    gb = g_sb[:100, c, :].unsqueeze(2).broadcast_to([100, DOCS_M, S])
                    nc.vector.tensor_tensor(
                        out=p_sb[:100, c, :].rearrange("p (b s) -> p b s", s=S),
                        in0=k_sb[:100, c, :].rearrange("p (b s) -> p b s", s=S),
                        in1=gb, op=MULT)

                # ---- beta logits from p ----
                eb_sb = act_pool.tile([6, MACRO], bft, tag="eb")
                for s in range(NSUB):
                    sl = slice(s * SUB, (s + 1) * SUB)
                    ps6 = ps_pool.tile([128, SUB], fp32, tag="mm")
                    for c in range(3):
                        nc.tensor.matmul(
                            ps6[:6, :], lhsT=wsb["ka6"][:100, c, :],
                            rhs=p_sb[:100, c, sl], start=(c == 0), stop=(c == 2))
                    nc.scalar.activation(out=eb_sb[:6, sl], in_=ps6[:6, :], func=AF.Exp)
                zb = small_pool.tile([6, DOCS_M], fp32, tag="zb")
                nc.vector.tensor_reduce(
                    out=zb[:], in_=eb_sb[:6, :].rearrange("p (b s) -> p b s", s=S),
                    axis=mybir.AxisListType.X, op=ADD)
                rzb = small_pool.tile([6, DOCS_M], fp32, tag="rzb")
                nc.vector.reciprocal(out=rzb[:], in_=zb[:])

                gk_un = small_pool.tile([128, 3, DOCS_M], fp32, tag="gkun")
                seg_wsum(gk_un, p_sb, 3, make_rep(eb_sb, "ebrep"), "b")
                gk_sb = small_pool.tile([128, 3, DOCS_M], bft, tag="gk")
                for c in range(3):
                    psZ = ps_s_pool.tile([128, DOCS_M], fp32, tag="mmz")
                    nc.tensor.matmul(
                        psZ[:100, :], lhsT=wsb["b6a_f"][:, c * 100:(c + 1) * 100],
                        rhs=rzb[:], start=True, stop=True)
                    nc.vector.tensor_tensor(
                        out=gk_sb[:100, c, :], in0=gk_un[:100, c, :],
                        in1=psZ[:100, :], op=MULT)

                # ---- u = gk (bcast) * v ----
                u_sb = act_pool.tile([128, 3, MACRO], bft, tag="u")
                for c in range(3):
                    gkb = gk_sb[:100, c, :].unsqueeze(2).broadcast_to([100, DOCS_M, S])
                    nc.vector.tensor_tensor(
                        out=u_sb[:100, c, :].rearrange("p (b s) -> p b s", s=S),
                        in0=v_sb[:100, c, :].rearrange("p (b s) -> p b s", s=S),
                        in1=gkb, op=MULT)

                if stages < 5:
                    continue
                # ---- h = u @ Wup + q @ Wqp + bh ----
                h_sb = act_pool.tile([128, 4, MACRO], bft, tag="h")
                z_sb = act_pool.tile([128, 2, MACRO], bft, tag="z")
                for s in range(NSUB):
                    sl = slice(s * SUB, (s + 1) * SUB)
                    for mo in range(4):
                        ps = ps_pool.tile([128, SUB], fp32, tag="mm")
                        for c in range(3):
                            nc.tensor.matmul(
                                ps[:100, :],
                                lhsT=wsb["wup"][:100, c, mo * 100:(mo + 1) * 100],
                                rhs=u_sb[:100, c, sl], start=(c == 0), stop=False)
                        for c in range(3):
                            nc.tensor.matmul(
                                ps[:100, :],
                                lhsT=wsb["wqp"][:100, c, mo * 100:(mo + 1) * 100],
                                rhs=q_sb[:100, c, sl], start=False, stop=(c == 2))
                        nc.scalar.activation(
                            out=h_sb[:100, mo, sl], in_=ps[:100, :],
                            func=AF.Identity, bias=wsb["bh"][:, mo:mo + 1])

                    # ---- z = tanh(h @ Wa + ba) ----
                    for mz in range(2):
                        ps = ps_pool.tile([128, SUB], fp32, tag="mm")
                        for c in range(4):
                            nc.tensor.matmul(
                                ps[:100, :],
                                lhsT=wsb["wa"][:100, c, mz * 100:(mz + 1) * 100],
                                rhs=h_sb[:100, c, sl], start=(c == 0), stop=(c == 3))
                        nc.scalar.activation(
                            out=z_sb[:100, mz, sl], in_=ps[:100, :], func=AF.Tanh,
                            bias=wsb["ba2"][:, mz:mz + 1])

                # ---- scores -> Es [1, MACRO] ----
                es_sb = act_pool.tile([1, MACRO], bft, tag="es")
                for s in range(NSUB):
                    sl = slice(s * SUB, (s + 1) * SUB)
                    psS = ps_pool.tile([128, SUB], fp32, tag="mm")
                    for mz in range(2):
                        nc.tensor.matmul(
                            psS[:1, :], lhsT=wsb["va2"][:100, mz, :],
                            rhs=z_sb[:100, mz, sl], start=(mz == 0), stop=(mz == 1))
                    nc.scalar.activation(out=es_sb[:1, sl], in_=psS[:1, :], func=AF.Exp)
                zs = small_pool.tile([1, DOCS_M], fp32, tag="zs")
                nc.vector.tensor_reduce(
                    out=zs[:], in_=es_sb[:1, :].rearrange("p (b s) -> p b s", s=S),
                    axis=mybir.AxisListType.X, op=ADD)
                rzs = small_pool.tile([1, DOCS_M], fp32, tag="rzs")
                nc.vector.reciprocal(out=rzs[:], in_=zs[:])

                # Es replicated across partitions (K=1 matmul with ones)
                esr = rep_pool.tile([128, MACRO], bft, tag="esrep")
                for s in range(NSUB):
                    sl = slice(s * SUB, (s + 1) * SUB)
                    psR = ps_pool.tile([128, SUB], fp32, tag="mm")
                    nc.tensor.matmul(
                        psR[:, :], lhsT=wsb["on1"][:, :], rhs=es_sb[:1, sl],
                        start=True, stop=True)
                    nc.scalar.activation(out=esr[:, sl], in_=psR[:, :], func=AF.Copy)

                if stages < 6:
                    continue
                # ---- pooled ----
                pl_un = small_pool.tile([128, 4, DOCS_M], fp32, tag="plun")
                seg_wsum(pl_un, h_sb, 4, lambda c: esr, "s")
                psRz = ps_s_pool.tile([128, DOCS_M], fp32, tag="mmz")
                nc.tensor.matmul(
                    psRz[:, :], lhsT=wsb["on1_f"][:, :], rhs=rzs[:], start=True,
                    stop=True)
                for c in range(4):
                    nc.vector.tensor_tensor(
                        out=pooled_acc[:100, c, m * DOCS_M:(m + 1) * DOCS_M],
                        in0=pl_un[:100, c, :], in1=psRz[:100, :], op=MULT)

            nc.sync.dma_start(out=outp[:100], in_=pooled_acc[:100])

    nc.compile()
    return nc


def _prepare_inputs(inputs):
    t = {k: np.asarray(v) for k, v in inputs.items()}
    tokens = np.asarray(t["tokens"], np.int64)

    emb_pad = np.zeros((V, EPAD), bf16)
    emb_pad[:, :D] = np.asarray(t["emb"], np.float32).astype(bf16)

    dev_w = _fold_weights(t)

    in_maps = []
    for core in range(NCORES):
        tk = tokens[core * BP:(core + 1) * BP].reshape(-1)   # [TOK]
        im = {"embp": emb_pad}
        idx = np.zeros((NMACRO, GIDX), np.int16)
        tkm = tk.reshape(NMACRO, MACRO)
        idx[:, :MACRO] = (tkm - REBASE).astype(np.int16)
        idx[:, MACRO:] = 0
        # wrap layout: value for gather-pos i goes to [i % 16, i // 16]
        idx_w = idx.reshape(NMACRO, GIDX // 16, 16).transpose(2, 0, 1).reshape(
            16, NMACRO * (GIDX // 16))
        im["idx"] = np.tile(idx_w, (8, 1))   # replicated per Q7 core group
        for nme, arr in dev_w.items():
            im[nme] = arr
        in_maps.append(im)
    return in_maps


def kernel(**inputs) -> np.ndarray:
    from concourse.bass_utils import run_bass_kernel_spmd

    if "nc" not in _CACHE:
        _CACHE["nc"] = _build_program()
    nc = _CACHE["nc"]

    in_maps = _prepare_inputs(inputs)
    kw = {}
    if os.environ.get("BASS_TRACE"):
        os.makedirs("/tmp/ktrace", exist_ok=True)
        kw = dict(tmpdir="/tmp/ktrace")
    res = run_bass_kernel_spmd(nc, in_maps, core_ids=list(range(NCORES)), **kw)
    _CACHE["last_results"] = res

    outs = []
    for core in range(NCORES):
        arr = np.asarray(res.results[core]["outp"])   # [128, 4, BP]
        pooled = arr[:100].transpose(1, 0, 2).reshape(E, BP).T   # [BP, 400]
        outs.append(pooled)
    return np.concatenate(outs, 0).astype(np.float32)


if __name__ == "__main__":
    import reference as ref
    inputs = ref.setup_inputs()
    out = kernel(**{k: np.asarray(v) for k, v in inputs.items()})
    print("out", out.shape, out.dtype)

